# revision 32
# baseline (speedup 1.0000x reference)
"""DSoftKI Trainium2 kernel (fp16 datapath).

Reference computation (per batch row b, interp point m, dim d; B=16384, M=512, D=8):
    diff[b,m,d] = x[b,d]/T[m,d] - z[m,d]
    dist[b,m]   = ||diff[b,m,:]||
    W           = softmax_m(-dist)
    dd          = diff / (dist+1e-6) / T          = numer * R
    mean_dd[b,d]= sum_m W*dd                      (acc)
    deriv       = -W*(dd - mean_dd)               = acc_d*W - numer_d*V
    out         = concat([W  (B rows) , deriv transposed to (b*D+d, m) rows])

with numer[b,m,d] = x[b,d]*A[m,d] - Bz[m,d],  A = 1/T^2, Bz = z/T,
     R = 1/dist, U = exp(-dist), S = sum_m U, W = U/S, V = W*R.

dist^2 comes from one PE matmul per 128-row tile (hi/lo bf16 split, K=66);
numer likewise is a K=6 bf16-split matmul per (tile, d).  The elementwise
pipeline runs in fp16 (error budget 2e-2 global-normalized; fp16 lands at
~6e-4): ACT computes the ln/exp chain (L, d2, U+S accum, W, R), GPSIMD
does V = W*R, DVE does the per-d work: t_d = numer_d*V via
scalar_tensor_tensor from PSUM (1x — PSUM fp32 operands disqualify the
DVE 2x/4x modes, measured) with accum_out giving acc_d for free; staging
tmp_d = acc_d*W rides ACT Copy for 6 d's and DVE tensor_scalar (4x mode,
~350ns) for 2; the deriv subtract is merged fp16 tensor_tensor (2x mode
on the 2048-wide half; the 1536-wide piece runs 1x - non-power-of-2
widths don't engage 2x - with the 512-wide tail on GPSIMD).  This exact
op/engine split is a measured local optimum: coarser or finer splits,
operand packing, and any extra GPSIMD queue traffic (it serializes V for
the next tile) all measured slower.  The deriv-finish phase is software-
pipelined one tile behind the stst phase so tile t's ACT stages queue
after tile t+1's ln/exp chain instead of head-of-line blocking it.
Outputs are written fp16 and upcast on the host.

Sharding: data-parallel over B across 8 cores (2048 rows each).
"""
import sys

sys.path.insert(0, "/opt/trn_rl_repo")

import numpy as np
import ml_dtypes

import concourse.bass as bass
import concourse.tile as tile
from concourse import bacc, mybir
from concourse.bass_utils import run_bass_kernel_spmd

dt = mybir.dt
AF = mybir.ActivationFunctionType
OP = mybir.AluOpType

B, M, D = 16384, 512, 8
N_CORES = 8
BSH = B // N_CORES          # 2048 rows per core
NT = BSH // 128             # 16 tiles of 128 rows
K_NUMER = 6                 # per-d numer matmul contraction rows
K_DIST = 8 * D + 2          # dist^2 matmul contraction rows (66)

NACT_STAGE = 6              # deriv stagings on ACT; rest via DVE tensor_scalar

_cache = {}


def _split_bf16(a):
    """fp32/fp64 array -> (hi, lo) bf16 pair with hi+lo ~ a to ~2^-16."""
    a = np.asarray(a, np.float32)
    hi = a.astype(ml_dtypes.bfloat16)
    lo = (a - hi.astype(np.float32)).astype(ml_dtypes.bfloat16)
    return hi, lo


def _force_single_act_table():
    """All activation funcs used here (ln, exp, copy) live in the
    'natural_log_exp_and_others' set; strip them from every other set so
    the table-load pass resolves a single resident set."""
    import concourse.bacc as _bacc
    from concourse.hw_specs import get_activation_tables as _orig

    def patched(arch):
        tabs = _orig(arch)
        keep = "natural_log_exp_and_others"
        strip = set()
        for f in ("Exp", "Ln", "Copy", "Identity", "MemsetZero", "Square",
                  "Abs", "Sign", "Relu", "Is_finite"):
            try:
                strip.add(getattr(mybir.ActivationFunctionType, f))
            except AttributeError:
                pass
        out = {}
        for name, funcs in tabs.items():
            out[name] = funcs if name == keep else (funcs - strip)
        return out

    _bacc.get_activation_tables = patched


def _build_program():
    _force_single_act_table()
    nc = bacc.Bacc("TRN2", target_bir_lowering=False, debug=False)

    xlhs_d = [nc.dram_tensor(f"xlhs{i}", [128, BSH], dt.bfloat16, kind="ExternalInput").ap()
              for i in range(4)]
    glhs_d = nc.dram_tensor("glhs", [K_DIST, BSH], dt.bfloat16, kind="ExternalInput").ap()
    hrhs_d = nc.dram_tensor("hrhs", [K_DIST, M], dt.bfloat16, kind="ExternalInput").ap()
    nrhs_d = [nc.dram_tensor(f"nrhs{i}", [128, M], dt.bfloat16, kind="ExternalInput").ap()
              for i in range(4)]
    w_d = nc.dram_tensor("w_out", [BSH, M], dt.float16, kind="ExternalOutput").ap()
    dv_d = nc.dram_tensor("d_out", [BSH * D, M], dt.float16, kind="ExternalOutput").ap()

    with tile.TileContext(nc) as tc:
        with tc.tile_pool(name="const", bufs=1) as cpool, \
             tc.tile_pool(name="work", bufs=4) as wpool, \
             tc.tile_pool(name="gbuf", bufs=4) as gpool, \
             tc.tile_pool(name="dbuf", bufs=3) as dpool, \
             tc.tile_pool(name="ps_s", bufs=3, space="PSUM") as ps_s, \
             tc.tile_pool(name="ps_n", bufs=5, space="PSUM") as ps_n:

            XLHS = [cpool.tile([128, BSH], dt.bfloat16, name=f"XLHS{i}", tag=f"xlhs{i}") for i in range(4)]
            GLHS = cpool.tile([K_DIST, BSH], dt.bfloat16)
            HRHS = cpool.tile([K_DIST, M], dt.bfloat16)
            NRHS = [cpool.tile([128, M], dt.bfloat16, name=f"NRHS{i}", tag=f"nrhs{i}") for i in range(4)]
            LN2 = cpool.tile([128, 1], dt.float32)
            nc.vector.memset(LN2[:], float(np.log(2.0)))
            # dist-matmul consts first so tile 0's s-matmul starts early;
            # then the head slices of the numer operands (tile 0/1 only needs
            # XLHS[:, 0:256]) so the first stst isn't stuck behind 2.5 MB of
            # input DMA; tails stream in afterwards.
            # parallel input-DMA dispatch: each dispatch costs ~0.65us of
            # queue time, so the numer operands are issued from the GPSIMD
            # and Vector queues (idle at startup) while Sync issues the
            # s-matmul constants; XLHS heads (tiles 0-3) land first.
            nc.sync.dma_start(HRHS[:], hrhs_d[:])
            nc.sync.dma_start(GLHS[:, 0:128], glhs_d[:, 0:128])
            nc.sync.dma_start(GLHS[:, 128:], glhs_d[:, 128:])
            for i in range(4):
                nc.gpsimd.dma_start(NRHS[i][:], nrhs_d[i][:])
            for i in range(4):
                nc.gpsimd.dma_start(XLHS[i][:, 0:512], xlhs_d[i][:, 0:512])
            for i in range(4):
                nc.sync.dma_start(XLHS[i][:, 512:], xlhs_d[i][:, 512:])

            dv_t = dv_d.rearrange("(t p d) m -> t p (d m)", p=128, d=D)
            w_t = w_d.rearrange("(t p) m -> t p m", p=128)

            # dist^2 matmuls are hoisted: s for tile t+1 is emitted in the
            # middle of tile t's numer-matmul stream, so the next tile's ACT
            # chain (-> W -> V -> stst) isn't serialized behind all 8 numers
            # on the PE queue.
            s_tiles = [None] * NT

            def emit_s(tt):
                sp = ps_s.tile([128, M], dt.float32, tag="s")
                nc.tensor.matmul(sp[:], GLHS[:, tt * 128:(tt + 1) * 128],
                                 HRHS[:], start=True, stop=True)
                s_tiles[tt] = sp

            def finish_deriv(fin):
                # deriv-finish for an earlier tile: stage tmp_d = acc_d*W
                # (6 ACT + 2 DVE tensor_scalar), merged fp16 subtracts, DMAs.
                ft, G, acc, W = fin
                tmp = dpool.tile([128, D * M], dt.float16, tag="tmp")
                for d in range(NACT_STAGE):
                    nc.scalar.activation(tmp[:, d * M:(d + 1) * M], W[:],
                                         AF.Copy, scale=acc[:, d:d + 1])
                for d in range(NACT_STAGE, D):
                    nc.vector.tensor_scalar(
                        tmp[:, d * M:(d + 1) * M], W[:], acc[:, d:d + 1], None,
                        op0=OP.mult)
                dvout = gpool.tile([128, D * M], dt.float16, tag="dvout")
                h = 4 * M
                h2 = 7 * M
                nc.vector.tensor_tensor(dvout[:, 0:h], tmp[:, 0:h],
                                        G[:, 0:h], op=OP.subtract)
                nc.sync.dma_start(dv_t[ft][:, 0:h], dvout[:, 0:h])
                nc.vector.tensor_tensor(dvout[:, h:h2], tmp[:, h:h2],
                                        G[:, h:h2], op=OP.subtract)
                nc.gpsimd.tensor_tensor(dvout[:, h2:], tmp[:, h2:],
                                        G[:, h2:], op=OP.subtract)
                nc.sync.dma_start(dv_t[ft][:, h:h2], dvout[:, h:h2])
                nc.sync.dma_start(dv_t[ft][:, h2:], dvout[:, h2:])

            pending = None
            emit_s(0)
            for t in range(NT):
                ts = slice(t * 128, (t + 1) * 128)
                s_ps = s_tiles[t]

                # --- ACT chain: L = ln s ; d2 = 2*dist = exp(.5L + ln2) ;
                # U = exp(-.5*d2) (accum -> S) ; W = exp(-.5*d2 + ln(1/S)) ;
                # R = 1/dist = exp(-.5L)
                L = wpool.tile([128, M], dt.float32, tag="L")
                nc.scalar.activation(L[:], s_ps[:], AF.Ln)
                d2 = wpool.tile([128, M], dt.float32, tag="d2")
                nc.scalar.activation(d2[:], L[:], AF.Exp, scale=0.5, bias=LN2[:])
                scr = wpool.tile([128, M], dt.float16, tag="scr")
                S = wpool.tile([128, 1], dt.float32, tag="S")
                nc.scalar.activation(scr[:], d2[:], AF.Exp, scale=-0.5, accum_out=S[:])

                nlnS = wpool.tile([128, 1], dt.float32, tag="nlnS")
                nc.scalar.activation(nlnS[:], S[:], AF.Ln)
                lninvS = wpool.tile([128, 1], dt.float32, tag="lninvS")
                nc.scalar.mul(lninvS[:], nlnS[:], -1.0)

                W = wpool.tile([128, M], dt.float16, tag="W")
                nc.scalar.activation(W[:], d2[:], AF.Exp, scale=-0.5, bias=lninvS[:])
                nc.sync.dma_start(w_t[t], W[:])
                R = wpool.tile([128, M], dt.float16, tag="R")
                nc.scalar.activation(R[:], L[:], AF.Exp, scale=-0.5)
                V = wpool.tile([128, M], dt.float16, tag="V")
                nc.gpsimd.tensor_tensor(V[:], W[:], R[:], op=OP.mult)

                # --- per-d: numer matmul ; t_d = numer*V (accum -> acc_d) ---
                G = gpool.tile([128, D * M], dt.float16, tag="G")
                acc = wpool.tile([128, D], dt.float32, tag="acc")
                for d in range(D):
                    np_ps = ps_n.tile([128, M], dt.float32, tag="n")
                    XL = XLHS[d // 2]
                    NRH = NRHS[d // 2]
                    p0 = 64 * (d % 2)
                    nc.tensor.matmul(
                        np_ps[:],
                        XL[p0:p0 + K_NUMER, ts],
                        NRH[p0:p0 + K_NUMER, :],
                        start=True, stop=True,
                    )
                    nc.vector.scalar_tensor_tensor(
                        G[:, d * M:(d + 1) * M], np_ps[:], 1.0, V[:],
                        op0=OP.mult, op1=OP.mult,
                        accum_out=acc[:, d:d + 1],
                    )
                    if d == 1 and t + 1 < NT:
                        emit_s(t + 1)

                # software pipeline: finish the PREVIOUS tile's deriv now, so
                # its ACT stages queue behind this tile's chain instead of
                # blocking it (head-of-line coupling measured in the trace).
                if pending is not None:
                    finish_deriv(pending)
                pending = (t, G, acc, W)
            finish_deriv(pending)

    nc.compile()
    return nc


def _host_prep(x, z, T):
    """Build per-core input maps.  All in fp64 for max const accuracy."""
    x64 = x.astype(np.float64)
    invT = 1.0 / T.astype(np.float64)          # [M, D]
    A = invT * invT
    Bz = z.astype(np.float64) * invT
    c = (z.astype(np.float64) ** 2).sum(axis=1)          # [M]

    Ah, Al = _split_bf16(A)                    # [M, D] each
    B2h, B2l = _split_bf16(-2.0 * Bz)
    Bnh, Bnl = _split_bf16(-Bz)
    ch, cl = _split_bf16(c)

    # dist rhs H [K_DIST, M]: groups x2h*(Ah,Al), x2l*(Ah,Al), xh*(B2h,B2l),
    # xl*(B2h,B2l), ones*(ch,cl)
    H = np.zeros((K_DIST, M), ml_dtypes.bfloat16)
    for d in range(D):
        H[0 * D + d] = Ah[:, d]
        H[1 * D + d] = Al[:, d]
        H[2 * D + d] = Ah[:, d]
        H[3 * D + d] = Al[:, d]
        H[4 * D + d] = B2h[:, d]
        H[5 * D + d] = B2l[:, d]
        H[6 * D + d] = B2h[:, d]
        H[7 * D + d] = B2l[:, d]
    H[8 * D] = ch
    H[8 * D + 1] = cl

    # numer rhs: four [128, M] tensors, d-blocks at partitions 0/64;
    # rows pair with lhsT rows [ones, ones, xh, xh, xl, xl] ->
    # [-Bzh, -Bzl, Ah, Al, Ah, Al]
    NRs = [np.zeros((128, M), ml_dtypes.bfloat16) for _ in range(4)]
    for d in range(D):
        NR = NRs[d // 2]
        b = 64 * (d % 2)
        NR[b + 0] = Bnh[:, d]
        NR[b + 1] = Bnl[:, d]
        NR[b + 2] = Ah[:, d]
        NR[b + 3] = Al[:, d]
        NR[b + 4] = Ah[:, d]
        NR[b + 5] = Al[:, d]

    in_maps = []
    for cix in range(N_CORES):
        xs = x64[cix * BSH:(cix + 1) * BSH]            # [BSH, D]
        xh, xl = _split_bf16(xs)
        x2h, x2l = _split_bf16(xs * xs)
        GL = np.zeros((K_DIST, BSH), ml_dtypes.bfloat16)
        for d in range(D):
            GL[0 * D + d] = x2h[:, d]
            GL[1 * D + d] = x2h[:, d]
            GL[2 * D + d] = x2l[:, d]
            GL[3 * D + d] = x2l[:, d]
            GL[4 * D + d] = xh[:, d]
            GL[5 * D + d] = xh[:, d]
            GL[6 * D + d] = xl[:, d]
            GL[7 * D + d] = xl[:, d]
        GL[8 * D] = 1.0
        GL[8 * D + 1] = 1.0
        XLs = [np.zeros((128, BSH), ml_dtypes.bfloat16) for _ in range(4)]
        for d in range(D):
            XL = XLs[d // 2]
            b = 64 * (d % 2)
            XL[b + 0] = 1.0
            XL[b + 1] = 1.0
            XL[b + 2] = xh[:, d]
            XL[b + 3] = xh[:, d]
            XL[b + 4] = xl[:, d]
            XL[b + 5] = xl[:, d]
        im = {"glhs": GL, "hrhs": H}
        for i in range(4):
            im[f"xlhs{i}"] = XLs[i]
            im[f"nrhs{i}"] = NRs[i]
        in_maps.append(im)
    return in_maps


def kernel(x, z, T, _trace=False):
    if "nc" not in _cache:
        _cache["nc"] = _build_program()
    nc = _cache["nc"]
    in_maps = _host_prep(np.asarray(x), np.asarray(z), np.asarray(T))
    res = run_bass_kernel_spmd(nc, in_maps, core_ids=list(range(N_CORES)), trace=_trace)
    _cache["last_exec_time_ns"] = res.exec_time_ns
    w_full = np.concatenate([r["w_out"] for r in res.results], axis=0).astype(np.float32)
    d_full = np.concatenate([r["d_out"] for r in res.results], axis=0).astype(np.float32)
    return np.concatenate([w_full, d_full], axis=0)
